# revision 1
# baseline (speedup 1.0000x reference)
"""Pairwise Euclidean distance kernel for Trainium2 (8 NeuronCores, SPMD).

Computes out[i, j] = ||mapping[i] - mapping[j]|| for mapping [8192, 512] fp32.

Strategy (row-sharded, data-parallel over query rows):
  - Host: cast mapping to bf16, transpose to T = bf16(A).T [512, 8192].
    Each core c gets the full T (rhs) plus -2*T[:, chunk_c] (lhsT weights)
    for its 1024-row chunk, so PE computes -2 * gram directly.
  - Row norms sq[i] = sum_d bf16(a_id)^2 are computed on host in fp32 from
    the *bf16-rounded* values; then d2 = sq_m + sq_n - 2*gram is the exact
    squared distance of the bf16-rounded points (>= 0 up to fp32 roundoff),
    which keeps the diagonal tight.
  - sq_n enters the PSUM accumulation as a K=2 bf16 matmul against a hi/lo
    split of sq (ones weights); sq_m is added per-partition by the DVE
    together with the relu clamp; ScalarE applies Sqrt; DMA to DRAM.
"""

import numpy as np
import ml_dtypes

N = 8192
D = 512
P = 128
NCORES = 8
CHUNK = N // NCORES            # rows per core
KT = D // P                    # k-tiles (4)
MT = CHUNK // P                # m-tiles per core (8)
NSUP = 2048                    # psum super-tile width (4 banks)
NSUB = 512                     # matmul free dim (1 bank)

_compiled = None               # cached (nc, meta)


def _build(n=N, d=D, chunk=CHUNK, nsup=NSUP):
    import concourse.mybir as mybir
    import concourse.tile as tile
    from concourse import bacc

    kt = d // P
    mt = chunk // P
    ns = n // nsup
    nb = nsup // NSUB

    nc = bacc.Bacc()
    tb_d = nc.dram_tensor("tb", [kt, P, n], mybir.dt.bfloat16, kind="ExternalInput")
    tbs_d = nc.dram_tensor("tbs", [kt, P, chunk], mybir.dt.bfloat16,
                           kind="ExternalInput")
    sq2_d = nc.dram_tensor("sq2", [2, n], mybir.dt.bfloat16, kind="ExternalInput")
    ones_d = nc.dram_tensor("ones2", [2, P], mybir.dt.bfloat16, kind="ExternalInput")
    sqc_d = nc.dram_tensor("sqc", [P, mt], mybir.dt.float32, kind="ExternalInput")
    out_d = nc.dram_tensor("out", [chunk, n], mybir.dt.float32,
                           kind="ExternalOutput")

    with tile.TileContext(nc) as tc:
        with (
            tc.tile_pool(name="const", bufs=1) as constp,
            tc.tile_pool(name="stage", bufs=4) as stagep,
            tc.tile_pool(name="psum", bufs=2, space="PSUM") as psump,
        ):
            tb = constp.tile([P, kt, n], mybir.dt.bfloat16, tag="tb")
            tbs = constp.tile([P, kt, chunk], mybir.dt.bfloat16, tag="tbs")
            sq2 = constp.tile([2, n], mybir.dt.bfloat16, tag="sq2")
            ones2 = constp.tile([2, P], mybir.dt.bfloat16, tag="ones2")
            sqc = constp.tile([P, mt], mybir.dt.float32, tag="sqc")

            for k in range(kt):
                nc.sync.dma_start(tb[:, k, :], tb_d[k])
                nc.sync.dma_start(tbs[:, k, :], tbs_d[k])
            nc.sync.dma_start(sq2[:], sq2_d[:])
            nc.sync.dma_start(ones2[:], ones_d[:])
            nc.sync.dma_start(sqc[:], sqc_d[:])

            for m in range(mt):
                for s in range(ns):
                    ps = psump.tile([P, nsup], mybir.dt.float32, tag="ps")
                    for b in range(nb):
                        n0 = s * nsup + b * NSUB
                        pslice = ps[:, b * NSUB:(b + 1) * NSUB]
                        for k in range(kt):
                            nc.tensor.matmul(
                                pslice,
                                tbs[:, k, m * P:(m + 1) * P],
                                tb[:, k, n0:n0 + NSUB],
                                start=(k == 0),
                                stop=False,
                            )
                        nc.tensor.matmul(
                            pslice,
                            ones2[:],
                            sq2[:, n0:n0 + NSUB],
                            start=False,
                            stop=True,
                        )
                    st = stagep.tile([P, nsup], mybir.dt.float32, tag="st")
                    nc.vector.tensor_scalar(
                        st[:], ps[:], sqc[:, m:m + 1], 0.0,
                        mybir.AluOpType.add, mybir.AluOpType.max,
                    )
                    nc.scalar.activation(
                        st[:], st[:], mybir.ActivationFunctionType.Sqrt,
                    )
                    nc.sync.dma_start(
                        out_d[m * P:(m + 1) * P, s * nsup:(s + 1) * nsup], st[:],
                    )

    nc.compile()
    return nc


def _prep_inputs(mapping, n=N, d=D, chunk=CHUNK, ncores=NCORES):
    """Shard + lay out host-side inputs for each core."""
    bf16 = ml_dtypes.bfloat16
    kt = d // P
    mt = chunk // P

    tbf = np.ascontiguousarray(mapping.T).astype(bf16)          # [d, n]
    tf32 = tbf.astype(np.float32)
    sq = np.sum(tf32 * tf32, axis=0, dtype=np.float32)          # [n]
    sq_hi = sq.astype(bf16)
    sq_lo = (sq - sq_hi.astype(np.float32)).astype(bf16)
    sq2 = np.stack([sq_hi, sq_lo], axis=0)                      # [2, n] bf16
    ones2 = np.ones((2, P), dtype=bf16)
    tb = np.ascontiguousarray(tbf.reshape(kt, P, n))

    in_maps = []
    for c in range(ncores):
        c0 = c * chunk
        tbs = np.ascontiguousarray(
            (tf32[:, c0:c0 + chunk] * -2.0).astype(bf16).reshape(kt, P, chunk))
        sqc = np.ascontiguousarray(
            sq[c0:c0 + chunk].reshape(mt, P).T)                 # [P, mt] fp32
        in_maps.append({
            "tb": tb,
            "tbs": tbs,
            "sq2": sq2,
            "ones2": ones2,
            "sqc": sqc.astype(np.float32),
        })
    return in_maps


def kernel(mapping: np.ndarray) -> np.ndarray:
    from concourse.bass_utils import run_bass_kernel_spmd

    global _compiled
    mapping = np.asarray(mapping, dtype=np.float32)
    assert mapping.shape == (N, D)
    if _compiled is None:
        _compiled = _build()
    nc = _compiled
    in_maps = _prep_inputs(mapping)
    res = run_bass_kernel_spmd(nc, in_maps, list(range(NCORES)))
    return np.concatenate([res.results[c]["out"] for c in range(NCORES)], axis=0)


# revision 3
# speedup vs baseline: 1.1942x; 1.1942x over previous
"""Pairwise Euclidean distance kernel for Trainium2 (8 NeuronCores, SPMD).

Computes out[i, j] = ||mapping[i] - mapping[j]|| for mapping [8192, 512] fp32.

Strategy (row-sharded, data-parallel over query rows):
  - Host: cast mapping to bf16, transpose to T = bf16(A).T [512, 8192].
    Each core c gets the full T (rhs) plus -2*T[:, chunk_c] (lhsT weights)
    for its 1024-row chunk, so PE computes -2 * gram directly.
  - Row norms sq[i] = sum_d bf16(a_id)^2 are computed on host in fp32 from
    the *bf16-rounded* values; then d2 = sq_m + sq_n - 2*gram is the exact
    squared distance of the bf16-rounded points (>= 0 up to fp32 roundoff),
    which keeps the diagonal tight.
  - sq_n enters the PSUM accumulation as a K=2 bf16 matmul against a hi/lo
    split of sq (ones weights); sq_m is added per-partition by the DVE
    together with the relu clamp; ScalarE applies Sqrt; DMA to DRAM.
"""

import numpy as np
import ml_dtypes

N = 8192
D = 512
P = 128
NCORES = 8
CHUNK = N // NCORES            # rows per core
KT = D // P                    # k-tiles (4)
MT = CHUNK // P                # m-tiles per core (8)
NSUP = 2048                    # psum super-tile width (4 banks)
NSUB = 512                     # matmul free dim (1 bank)

_compiled = None               # cached (nc, meta)


def _build(n=N, d=D, chunk=CHUNK, nsup=NSUP):
    import concourse.mybir as mybir
    import concourse.tile as tile
    from concourse import bacc

    kt = d // P
    mt = chunk // P
    ns = n // nsup
    nb = nsup // NSUB

    nc = bacc.Bacc()
    tb_d = nc.dram_tensor("tb", [kt, P, n], mybir.dt.bfloat16, kind="ExternalInput")
    tbs_d = nc.dram_tensor("tbs", [kt, P, chunk], mybir.dt.bfloat16,
                           kind="ExternalInput")
    sq2_d = nc.dram_tensor("sq2", [2, n], mybir.dt.bfloat16, kind="ExternalInput")
    ones_d = nc.dram_tensor("ones2", [2, P], mybir.dt.bfloat16, kind="ExternalInput")
    sqc_d = nc.dram_tensor("sqc", [P, mt], mybir.dt.float32, kind="ExternalInput")
    out_d = nc.dram_tensor("out", [chunk, n], mybir.dt.float32,
                           kind="ExternalOutput")

    with tile.TileContext(nc) as tc:
        with (
            tc.tile_pool(name="const", bufs=1) as constp,
            tc.tile_pool(name="stage", bufs=4) as stagep,
            tc.tile_pool(name="psum", bufs=2, space="PSUM") as psump,
        ):
            tb = constp.tile([P, kt, n], mybir.dt.bfloat16, tag="tb")
            tbs = constp.tile([P, kt, chunk], mybir.dt.bfloat16, tag="tbs")
            sq2 = constp.tile([2, n], mybir.dt.bfloat16, tag="sq2")
            ones2 = constp.tile([2, P], mybir.dt.bfloat16, tag="ones2")
            sqc = constp.tile([P, mt], mybir.dt.float32, tag="sqc")

            nc.sync.dma_start(sq2[:], sq2_d[:])
            nc.sync.dma_start(ones2[:], ones_d[:])
            nc.sync.dma_start(sqc[:], sqc_d[:])
            for k in range(kt):
                nc.sync.dma_start(tbs[:, k, :], tbs_d[k])
            # Split tb loads by n-super so the first matmul group's inputs
            # arrive quickly instead of after 8MB of DMA.
            for s in range(ns):
                for k in range(kt):
                    nc.sync.dma_start(
                        tb[:, k, s * nsup:(s + 1) * nsup],
                        tb_d[k, :, s * nsup:(s + 1) * nsup],
                    )

            for m in range(mt):
                for s in range(ns):
                    ps = psump.tile([P, nsup], mybir.dt.float32, tag="ps")
                    # k outer / bank inner: 4 consecutive matmuls share the
                    # same stationary weights, letting them pipeline instead
                    # of serializing on per-matmul weight loads.
                    for k in range(kt):
                        for b in range(nb):
                            n0 = s * nsup + b * NSUB
                            nc.tensor.matmul(
                                ps[:, b * NSUB:(b + 1) * NSUB],
                                tbs[:, k, m * P:(m + 1) * P],
                                tb[:, k, n0:n0 + NSUB],
                                start=(k == 0),
                                stop=False,
                            )
                    for b in range(nb):
                        n0 = s * nsup + b * NSUB
                        nc.tensor.matmul(
                            ps[:, b * NSUB:(b + 1) * NSUB],
                            ones2[:],
                            sq2[:, n0:n0 + NSUB],
                            start=False,
                            stop=True,
                        )
                    st = stagep.tile([P, nsup], mybir.dt.float32, tag="st")
                    nc.vector.tensor_scalar(
                        st[:], ps[:], sqc[:, m:m + 1], 0.0,
                        mybir.AluOpType.add, mybir.AluOpType.max,
                    )
                    nc.scalar.activation(
                        st[:], st[:], mybir.ActivationFunctionType.Sqrt,
                    )
                    nc.sync.dma_start(
                        out_d[m * P:(m + 1) * P, s * nsup:(s + 1) * nsup], st[:],
                    )

    nc.compile()
    return nc


def _prep_inputs(mapping, n=N, d=D, chunk=CHUNK, ncores=NCORES):
    """Shard + lay out host-side inputs for each core."""
    bf16 = ml_dtypes.bfloat16
    kt = d // P
    mt = chunk // P

    tbf = np.ascontiguousarray(mapping.T).astype(bf16)          # [d, n]
    tf32 = tbf.astype(np.float32)
    sq = np.sum(tf32 * tf32, axis=0, dtype=np.float32)          # [n]
    sq_hi = sq.astype(bf16)
    sq_lo = (sq - sq_hi.astype(np.float32)).astype(bf16)
    sq2 = np.stack([sq_hi, sq_lo], axis=0)                      # [2, n] bf16
    ones2 = np.ones((2, P), dtype=bf16)
    tb = np.ascontiguousarray(tbf.reshape(kt, P, n))

    in_maps = []
    for c in range(ncores):
        c0 = c * chunk
        tbs = np.ascontiguousarray(
            (tf32[:, c0:c0 + chunk] * -2.0).astype(bf16).reshape(kt, P, chunk))
        sqc = np.ascontiguousarray(
            sq[c0:c0 + chunk].reshape(mt, P).T)                 # [P, mt] fp32
        in_maps.append({
            "tb": tb,
            "tbs": tbs,
            "sq2": sq2,
            "ones2": ones2,
            "sqc": sqc.astype(np.float32),
        })
    return in_maps


def kernel(mapping: np.ndarray) -> np.ndarray:
    from concourse.bass_utils import run_bass_kernel_spmd

    global _compiled
    mapping = np.asarray(mapping, dtype=np.float32)
    assert mapping.shape == (N, D)
    if _compiled is None:
        _compiled = _build()
    nc = _compiled
    in_maps = _prep_inputs(mapping)
    res = run_bass_kernel_spmd(nc, in_maps, list(range(NCORES)))
    return np.concatenate([res.results[c]["out"] for c in range(NCORES)], axis=0)


# revision 9
# speedup vs baseline: 1.2072x; 1.0109x over previous
"""Pairwise Euclidean distance kernel for Trainium2 (8 NeuronCores, SPMD).

Computes out[i, j] = ||mapping[i] - mapping[j]|| for mapping [8192, 512] fp32.

Strategy (row-sharded, data-parallel over query rows):
  - Host: cast mapping to bf16, transpose to T = bf16(A).T [512, 8192].
    Each core c gets the full T (rhs) plus -2*T[:, chunk_c] (lhsT weights)
    for its 1024-row chunk, so PE computes -2 * gram directly.
  - Row norms sq[i] = sum_d bf16(a_id)^2 are computed on host in fp32 from
    the *bf16-rounded* values; then d2 = sq_m + sq_n - 2*gram is the exact
    squared distance of the bf16-rounded points (>= 0 up to fp32 roundoff),
    which keeps the diagonal tight.
  - sq_n enters the PSUM accumulation as a K=2 bf16 matmul against a hi/lo
    split of sq (ones weights); sq_m is added per-partition by the DVE
    together with the relu clamp; ScalarE applies Sqrt; DMA to DRAM.
"""

import numpy as np
import ml_dtypes


def _dedup_ldweights(nc):
    """Remove back-to-back redundant weight loads.

    Tile legalization splits every matmul into LDWEIGHTS + MATMUL even when a
    run of matmuls shares one stationary operand; the redundant loads carry no
    semaphore waits/updates but serialize the PE array (each reload must wait
    for the prior matmul to drain). Dropping them lets same-weight matmuls
    stream back-to-back. Only loads with empty sync_info and a signature
    identical to the previous load are removed; any transpose-mode matmul or
    differing load resets the tracked state.
    """
    import concourse.mybir as mybir

    def sig(ldw):
        w = ldw.ins[0]
        return (w.memref, w.offset, str(w.ap), str(w.dtype),
                str(getattr(ldw, "perf_mode", None)),
                str(getattr(ldw, "is_transpose", None)),
                str(getattr(ldw, "tile_position", None)))

    removed = 0
    for f in nc.m.functions:
        for blk in f.blocks:
            last = None
            keep = []
            for inst in blk.instructions:
                if isinstance(inst, mybir.InstLdweights):
                    si = inst.sync_info
                    clean = si is None or (not si.on_wait and not si.on_update)
                    s = sig(inst)
                    if clean and last is not None and s == last:
                        removed += 1
                        continue
                    last = s
                elif isinstance(inst, mybir.InstMatmult):
                    if getattr(inst, "is_transpose", None):
                        last = None
                keep.append(inst)
            blk.instructions[:] = keep
    return removed


N = 8192
D = 512
P = 128
NCORES = 8
CHUNK = N // NCORES            # rows per core
KT = D // P                    # k-tiles (4)
MT = CHUNK // P                # m-tiles per core (8)
NSUP = 2048                    # psum super-tile width (4 banks)
NSUB = 512                     # matmul free dim (1 bank)

_compiled = None               # cached (nc, meta)


def _build(n=N, d=D, chunk=CHUNK, nsup=NSUP):
    import concourse.mybir as mybir
    import concourse.tile as tile
    from concourse import bacc

    kt = d // P
    mt = chunk // P
    ns = n // nsup
    nb = nsup // NSUB

    nc = bacc.Bacc()
    tb_d = nc.dram_tensor("tb", [kt, P, n], mybir.dt.bfloat16, kind="ExternalInput")
    tbs_d = nc.dram_tensor("tbs", [kt, P, chunk], mybir.dt.bfloat16,
                           kind="ExternalInput")
    sq2_d = nc.dram_tensor("sq2", [2, n], mybir.dt.bfloat16, kind="ExternalInput")
    ones_d = nc.dram_tensor("ones2", [2, P], mybir.dt.bfloat16, kind="ExternalInput")
    sqc_d = nc.dram_tensor("sqc", [P, mt], mybir.dt.float32, kind="ExternalInput")
    out_d = nc.dram_tensor("out", [chunk, n], mybir.dt.float32,
                           kind="ExternalOutput")

    with tile.TileContext(nc) as tc:
        with (
            tc.tile_pool(name="const", bufs=1) as constp,
            tc.tile_pool(name="stage", bufs=4) as stagep,
            tc.tile_pool(name="psum", bufs=2, space="PSUM") as psump,
        ):
            tb = constp.tile([P, kt, n], mybir.dt.bfloat16, tag="tb")
            tbs = constp.tile([P, kt, chunk], mybir.dt.bfloat16, tag="tbs")
            sq2 = constp.tile([2, n], mybir.dt.bfloat16, tag="sq2")
            ones2 = constp.tile([2, P], mybir.dt.bfloat16, tag="ones2")
            sqc = constp.tile([P, mt], mybir.dt.float32, tag="sqc")

            nc.sync.dma_start(sq2[:], sq2_d[:])
            nc.sync.dma_start(ones2[:], ones_d[:])
            nc.sync.dma_start(sqc[:], sqc_d[:])
            for k in range(kt):
                nc.sync.dma_start(tbs[:, k, :], tbs_d[k])
            # Split tb loads by n-super so the first matmul group's inputs
            # arrive quickly instead of after 8MB of DMA.
            for s in range(ns):
                for k in range(kt):
                    nc.sync.dma_start(
                        tb[:, k, s * nsup:(s + 1) * nsup],
                        tb_d[k, :, s * nsup:(s + 1) * nsup],
                    )

            for m in range(mt):
                for s in range(ns):
                    ps = psump.tile([P, nsup], mybir.dt.float32, tag="ps")
                    # k outer / bank inner: 4 consecutive matmuls share the
                    # same stationary weights, letting them pipeline instead
                    # of serializing on per-matmul weight loads.
                    for k in range(kt):
                        for b in range(nb):
                            n0 = s * nsup + b * NSUB
                            nc.tensor.matmul(
                                ps[:, b * NSUB:(b + 1) * NSUB],
                                tbs[:, k, m * P:(m + 1) * P],
                                tb[:, k, n0:n0 + NSUB],
                                start=(k == 0),
                                stop=False,
                            )
                    for b in range(nb):
                        n0 = s * nsup + b * NSUB
                        nc.tensor.matmul(
                            ps[:, b * NSUB:(b + 1) * NSUB],
                            ones2[:],
                            sq2[:, n0:n0 + NSUB],
                            start=False,
                            stop=True,
                        )
                    st = stagep.tile([P, nsup], mybir.dt.float32, tag="st")
                    nc.vector.tensor_scalar(
                        st[:], ps[:], sqc[:, m:m + 1], 0.0,
                        mybir.AluOpType.add, mybir.AluOpType.max,
                    )
                    nc.scalar.activation(
                        st[:], st[:], mybir.ActivationFunctionType.Sqrt,
                    )
                    nc.sync.dma_start(
                        out_d[m * P:(m + 1) * P, s * nsup:(s + 1) * nsup], st[:],
                    )

    nc.compile()
    _dedup_ldweights(nc)
    return nc


def _prep_inputs(mapping, n=N, d=D, chunk=CHUNK, ncores=NCORES):
    """Shard + lay out host-side inputs for each core."""
    bf16 = ml_dtypes.bfloat16
    kt = d // P
    mt = chunk // P

    tbf = np.ascontiguousarray(mapping.T).astype(bf16)          # [d, n]
    tf32 = tbf.astype(np.float32)
    sq = np.sum(tf32 * tf32, axis=0, dtype=np.float32)          # [n]
    sq_hi = sq.astype(bf16)
    sq_lo = (sq - sq_hi.astype(np.float32)).astype(bf16)
    sq2 = np.stack([sq_hi, sq_lo], axis=0)                      # [2, n] bf16
    ones2 = np.ones((2, P), dtype=bf16)
    tb = np.ascontiguousarray(tbf.reshape(kt, P, n))

    in_maps = []
    for c in range(ncores):
        c0 = c * chunk
        tbs = np.ascontiguousarray(
            (tf32[:, c0:c0 + chunk] * -2.0).astype(bf16).reshape(kt, P, chunk))
        sqc = np.ascontiguousarray(
            sq[c0:c0 + chunk].reshape(mt, P).T)                 # [P, mt] fp32
        in_maps.append({
            "tb": tb,
            "tbs": tbs,
            "sq2": sq2,
            "ones2": ones2,
            "sqc": sqc.astype(np.float32),
        })
    return in_maps


def kernel(mapping: np.ndarray) -> np.ndarray:
    from concourse.bass_utils import run_bass_kernel_spmd

    global _compiled
    mapping = np.asarray(mapping, dtype=np.float32)
    assert mapping.shape == (N, D)
    if _compiled is None:
        _compiled = _build()
    nc = _compiled
    in_maps = _prep_inputs(mapping)
    res = run_bass_kernel_spmd(nc, in_maps, list(range(NCORES)))
    return np.concatenate([res.results[c]["out"] for c in range(NCORES)], axis=0)


# revision 10
# speedup vs baseline: 1.6023x; 1.3273x over previous
"""Pairwise Euclidean distance kernel for Trainium2 (8 NeuronCores, SPMD).

Computes out[i, j] = ||mapping[i] - mapping[j]|| for mapping [8192, 512] fp32.

Strategy: symmetric (triangular) block decomposition, data-parallel and
perfectly load-balanced across cores.

  - The 8192 rows form 16 stripes of 512. Stripe s only computes columns
    from 2048*(s//4) upward (a 2048-aligned cover of the upper triangle),
    i.e. 4 - s//4 column blocks of [512 x 2048]. Pairing stripes (c, 15-c)
    gives every core exactly 5 such jobs. The strictly-lower-triangle
    remainder is mirrored from the transpose on the host (<5% of the matrix
    is computed redundantly).
  - Host casts mapping to bf16 and transposes to T = bf16(A).T [512, 8192].
    Per job the kernel gets lhsT = -2*T[:, rows] (weights) and rhs =
    T[:, cols], so PE accumulates -2*gram into PSUM. Row norms
    sq[i] = sum_d bf16(a_id)^2 are computed on the host in fp32 from the
    bf16-rounded values, making d2 = sq_m + sq_n - 2*gram the exact squared
    distance of the bf16-rounded points (>= -eps), which keeps the diagonal
    tight. sq_n joins the PSUM accumulation as a K=2 bf16 matmul against a
    hi/lo split of sq (ones weights); sq_m is added per-partition by the
    DVE fused with the relu clamp; ScalarE applies Sqrt; DMA out.
  - A post-compile pass drops back-to-back redundant LDWEIGHTS so runs of
    matmuls sharing one stationary operand pipeline on the PE array.
"""

import numpy as np
import ml_dtypes

N = 8192
D = 512
P = 128
NCORES = 8
NSTRIPES = 16
SW = N // NSTRIPES             # stripe width (512 rows)
NSUP = 2048                    # job col width / psum super-tile (4 banks)
NSUB = 512                     # matmul free dim (1 bank)
KT = D // P                    # k-tiles (4)
MT = SW // P                   # m-tiles per stripe (4)
NB = NSUP // NSUB              # banks per job (4)
NJOBS = 5                      # [512 x 2048] jobs per core

_compiled = None


def _jobs_for_core(c):
    """Five (stripe, col_block) jobs; col_block indexes 2048-wide blocks."""
    jobs = []
    for s in (c, NSTRIPES - 1 - c):
        for b in range(s // 4, 4):
            jobs.append((s, b))
    assert len(jobs) == NJOBS
    return jobs


def _dedup_ldweights(nc):
    """Remove back-to-back redundant weight loads.

    Tile legalization splits every matmul into LDWEIGHTS + MATMUL even when a
    run of matmuls shares one stationary operand; the redundant loads carry no
    semaphore waits/updates but serialize the PE array (each reload must wait
    for the prior matmul to drain). Dropping them lets same-weight matmuls
    stream back-to-back. Only loads with empty sync_info and a signature
    identical to the previous load are removed; any transpose-mode matmul or
    differing load resets the tracked state.
    """
    import concourse.mybir as mybir

    def sig(ldw):
        w = ldw.ins[0]
        return (w.memref, w.offset, str(w.ap), str(w.dtype),
                str(getattr(ldw, "perf_mode", None)),
                str(getattr(ldw, "is_transpose", None)),
                str(getattr(ldw, "tile_position", None)))

    removed = 0
    for f in nc.m.functions:
        for blk in f.blocks:
            last = None
            keep = []
            for inst in blk.instructions:
                if isinstance(inst, mybir.InstLdweights):
                    si = inst.sync_info
                    clean = si is None or (not si.on_wait and not si.on_update)
                    s = sig(inst)
                    if clean and last is not None and s == last:
                        removed += 1
                        continue
                    last = s
                elif isinstance(inst, mybir.InstMatmult):
                    if getattr(inst, "is_transpose", None):
                        last = None
                keep.append(inst)
            blk.instructions[:] = keep
    return removed


def _build():
    import concourse.mybir as mybir
    import concourse.tile as tile
    from concourse import bacc

    nc = bacc.Bacc()
    rhs_d = nc.dram_tensor("rhs", [KT, P, NJOBS, NSUP], mybir.dt.bfloat16,
                           kind="ExternalInput")
    lhs_d = nc.dram_tensor("lhs", [KT, P, NJOBS, SW], mybir.dt.bfloat16,
                           kind="ExternalInput")
    sq2_d = nc.dram_tensor("sq2", [2, NJOBS, NSUP], mybir.dt.bfloat16,
                           kind="ExternalInput")
    ones_d = nc.dram_tensor("ones2", [2, P], mybir.dt.bfloat16,
                            kind="ExternalInput")
    sqc_d = nc.dram_tensor("sqc", [P, NJOBS, MT], mybir.dt.float32,
                           kind="ExternalInput")
    out_d = nc.dram_tensor("out", [NJOBS, SW, NSUP], mybir.dt.float32,
                           kind="ExternalOutput")

    with tile.TileContext(nc) as tc:
        with (
            tc.tile_pool(name="const", bufs=1) as constp,
            tc.tile_pool(name="stage", bufs=4) as stagep,
            tc.tile_pool(name="psum", bufs=2, space="PSUM") as psump,
        ):
            rhs = constp.tile([P, KT, NJOBS, NSUP], mybir.dt.bfloat16, tag="rhs")
            lhs = constp.tile([P, KT, NJOBS, SW], mybir.dt.bfloat16, tag="lhs")
            sq2 = constp.tile([2, NJOBS, NSUP], mybir.dt.bfloat16, tag="sq2")
            ones2 = constp.tile([2, P], mybir.dt.bfloat16, tag="ones2")
            sqc = constp.tile([P, NJOBS, MT], mybir.dt.float32, tag="sqc")

            nc.sync.dma_start(sq2[:], sq2_d[:])
            nc.sync.dma_start(ones2[:], ones_d[:])
            nc.sync.dma_start(sqc[:], sqc_d[:])
            for k in range(KT):
                nc.sync.dma_start(lhs[:, k], lhs_d[k])
            # rhs loads split per (job, k) so job 0 can start after ~2MB.
            for j in range(NJOBS):
                for k in range(KT):
                    nc.sync.dma_start(rhs[:, k, j], rhs_d[k, :, j])

            for j in range(NJOBS):
                for m in range(MT):
                    ps = psump.tile([P, NSUP], mybir.dt.float32, tag="ps")
                    # k outer / bank inner: 4 consecutive matmuls share one
                    # stationary operand and pipeline after LDW dedup.
                    for k in range(KT):
                        for b in range(NB):
                            nc.tensor.matmul(
                                ps[:, b * NSUB:(b + 1) * NSUB],
                                lhs[:, k, j, m * P:(m + 1) * P],
                                rhs[:, k, j, b * NSUB:(b + 1) * NSUB],
                                start=(k == 0),
                                stop=False,
                            )
                    for b in range(NB):
                        nc.tensor.matmul(
                            ps[:, b * NSUB:(b + 1) * NSUB],
                            ones2[:],
                            sq2[:, j, b * NSUB:(b + 1) * NSUB],
                            start=False,
                            stop=True,
                        )
                    st = stagep.tile([P, NSUP], mybir.dt.float32, tag="st")
                    nc.vector.tensor_scalar(
                        st[:], ps[:], sqc[:, j, m:m + 1], 0.0,
                        mybir.AluOpType.add, mybir.AluOpType.max,
                    )
                    nc.scalar.activation(
                        st[:], st[:], mybir.ActivationFunctionType.Sqrt,
                    )
                    nc.sync.dma_start(out_d[j, m * P:(m + 1) * P, :], st[:])

    nc.compile()
    _dedup_ldweights(nc)
    return nc


def _prep_inputs(mapping):
    """Host-side shard/layout: per-core concatenated job operands."""
    bf16 = ml_dtypes.bfloat16

    tbf = np.ascontiguousarray(mapping.T).astype(bf16)          # [D, N]
    tf32 = tbf.astype(np.float32)
    tneg = (tf32 * -2.0).astype(bf16)                           # exact -2x
    sq = np.sum(tf32 * tf32, axis=0, dtype=np.float32)          # [N]
    sq_hi = sq.astype(bf16)
    sq_lo = (sq - sq_hi.astype(np.float32)).astype(bf16)
    ones2 = np.ones((2, P), dtype=bf16)

    tbf_k = tbf.reshape(KT, P, N)
    tneg_k = tneg.reshape(KT, P, N)

    in_maps = []
    for c in range(NCORES):
        jobs = _jobs_for_core(c)
        rhs = np.empty((KT, P, NJOBS, NSUP), dtype=bf16)
        lhs = np.empty((KT, P, NJOBS, SW), dtype=bf16)
        sq2 = np.empty((2, NJOBS, NSUP), dtype=bf16)
        sqc = np.empty((P, NJOBS, MT), dtype=np.float32)
        for j, (s, b) in enumerate(jobs):
            rhs[:, :, j, :] = tbf_k[:, :, b * NSUP:(b + 1) * NSUP]
            lhs[:, :, j, :] = tneg_k[:, :, s * SW:(s + 1) * SW]
            sq2[0, j] = sq_hi[b * NSUP:(b + 1) * NSUP]
            sq2[1, j] = sq_lo[b * NSUP:(b + 1) * NSUP]
            sqc[:, j, :] = sq[s * SW:(s + 1) * SW].reshape(MT, P).T
        in_maps.append({
            "rhs": rhs, "lhs": lhs, "sq2": sq2, "ones2": ones2, "sqc": sqc,
        })
    return in_maps


def _assemble(results):
    """Scatter per-core job blocks and mirror the lower triangle."""
    out = np.empty((N, N), dtype=np.float32)
    for c in range(NCORES):
        blocks = results[c]["out"]                              # [NJOBS, SW, NSUP]
        for j, (s, b) in enumerate(_jobs_for_core(c)):
            out[s * SW:(s + 1) * SW, b * NSUP:(b + 1) * NSUP] = blocks[j]
    # rows of stripe s below the 2048-aligned cover come from the transpose
    for s in range(NSTRIPES):
        c0 = (s // 4) * NSUP
        if c0:
            out[s * SW:(s + 1) * SW, :c0] = out[:c0, s * SW:(s + 1) * SW].T
    return out


def kernel(mapping: np.ndarray) -> np.ndarray:
    from concourse.bass_utils import run_bass_kernel_spmd

    global _compiled
    mapping = np.asarray(mapping, dtype=np.float32)
    assert mapping.shape == (N, D)
    if _compiled is None:
        _compiled = _build()
    in_maps = _prep_inputs(mapping)
    res = run_bass_kernel_spmd(_compiled, in_maps, list(range(NCORES)))
    return _assemble(res.results)


# revision 14
# speedup vs baseline: 1.8406x; 1.1487x over previous
"""Pairwise Euclidean distance kernel for Trainium2 (8 NeuronCores, SPMD).

Computes out[i, j] = ||mapping[i] - mapping[j]|| for mapping [8192, 512] fp32.

Strategy: symmetric (triangular) block decomposition, data-parallel and
perfectly load-balanced across cores.

  - The 8192 rows form 16 stripes of 512. Stripe s only computes columns
    from 2048*(s//4) upward (a 2048-aligned cover of the upper triangle),
    i.e. 4 - s//4 column blocks of [512 x 2048]. Pairing stripes (c, 15-c)
    gives every core exactly 5 such jobs. The strictly-lower-triangle
    remainder is mirrored from the transpose on the host (<5% of the matrix
    is computed redundantly).
  - Host casts mapping to bf16 and transposes to T = bf16(A).T [512, 8192].
    Per job the kernel gets lhsT = -2*T[:, rows] (weights) and rhs =
    T[:, cols], so PE accumulates -2*gram into PSUM. Row norms
    sq[i] = sum_d bf16(a_id)^2 are computed on the host in fp32 from the
    bf16-rounded values, making d2 = sq_m + sq_n - 2*gram the exact squared
    distance of the bf16-rounded points (>= -eps), which keeps the diagonal
    tight. sq_n joins the PSUM accumulation as a K=2 bf16 matmul against a
    hi/lo split of sq (ones weights); sq_m is added per-partition by the
    DVE fused with the relu clamp; ScalarE applies Sqrt; DMA out.
  - A post-compile pass drops back-to-back redundant LDWEIGHTS so runs of
    matmuls sharing one stationary operand pipeline on the PE array.
"""

import numpy as np
import ml_dtypes

N = 8192
D = 512
P = 128
NCORES = 8
NSTRIPES = 16
SW = N // NSTRIPES             # stripe width (512 rows)
NSUP = 2048                    # job col width / psum super-tile (4 banks)
NSUB = 512                     # matmul free dim (1 bank)
KT = D // P                    # k-tiles (4)
MT = SW // P                   # m-tiles per stripe (4)
NB = NSUP // NSUB              # banks per job (4)
NJOBS = 5                      # [512 x 2048] jobs per core

_compiled = None


def _jobs_for_core(c):
    """Five (stripe, col_block) jobs; col_block indexes 2048-wide blocks."""
    jobs = []
    for s in (c, NSTRIPES - 1 - c):
        for b in range(s // 4, 4):
            jobs.append((s, b))
    assert len(jobs) == NJOBS
    return jobs


def _dedup_ldweights(nc):
    """Remove back-to-back redundant weight loads.

    Tile legalization splits every matmul into LDWEIGHTS + MATMUL even when a
    run of matmuls shares one stationary operand; the redundant loads carry no
    semaphore waits/updates but serialize the PE array (each reload must wait
    for the prior matmul to drain). Dropping them lets same-weight matmuls
    stream back-to-back. Only loads with empty sync_info and a signature
    identical to the previous load are removed; any transpose-mode matmul or
    differing load resets the tracked state.
    """
    import concourse.mybir as mybir

    def sig(ldw):
        w = ldw.ins[0]
        return (w.memref, w.offset, str(w.ap), str(w.dtype),
                str(getattr(ldw, "perf_mode", None)),
                str(getattr(ldw, "is_transpose", None)),
                str(getattr(ldw, "tile_position", None)))

    removed = 0
    for f in nc.m.functions:
        for blk in f.blocks:
            last = None
            keep = []
            for inst in blk.instructions:
                if isinstance(inst, mybir.InstLdweights):
                    si = inst.sync_info
                    clean = si is None or (not si.on_wait and not si.on_update)
                    s = sig(inst)
                    if clean and last is not None and s == last:
                        removed += 1
                        continue
                    last = s
                elif isinstance(inst, mybir.InstMatmult):
                    if getattr(inst, "is_transpose", None):
                        last = None
                keep.append(inst)
            blk.instructions[:] = keep
    return removed


def _build():
    import concourse.mybir as mybir
    import concourse.tile as tile
    from concourse import bacc

    nc = bacc.Bacc()
    rhs_d = nc.dram_tensor("rhs", [KT, P, NJOBS, NSUP], mybir.dt.bfloat16,
                           kind="ExternalInput")
    lhs_d = nc.dram_tensor("lhs", [KT, P, NJOBS, SW], mybir.dt.bfloat16,
                           kind="ExternalInput")
    sq2_d = nc.dram_tensor("sq2", [2, NJOBS, NSUP], mybir.dt.bfloat16,
                           kind="ExternalInput")
    ones_d = nc.dram_tensor("ones2", [2, P], mybir.dt.bfloat16,
                            kind="ExternalInput")
    sqc_d = nc.dram_tensor("sqc", [P, NJOBS, MT], mybir.dt.float32,
                           kind="ExternalInput")
    # Output travels as bf16 (halves HBM write + host download traffic);
    # the host upcasts to fp32. d2 stays fp32 through the whole pipeline —
    # only the final sqrt result is rounded.
    out_d = nc.dram_tensor("out", [NJOBS, SW, NSUP], mybir.dt.bfloat16,
                           kind="ExternalOutput")

    with tile.TileContext(nc) as tc:
        with (
            tc.tile_pool(name="const", bufs=1) as constp,
            tc.tile_pool(name="stage", bufs=4) as stagep,
            tc.tile_pool(name="psum", bufs=2, space="PSUM") as psump,
        ):
            rhs = constp.tile([P, KT, NJOBS, NSUP], mybir.dt.bfloat16, tag="rhs")
            lhs = constp.tile([P, KT, NJOBS, SW], mybir.dt.bfloat16, tag="lhs")
            sq2 = constp.tile([2, NJOBS, NSUP], mybir.dt.bfloat16, tag="sq2")
            ones2 = constp.tile([2, P], mybir.dt.bfloat16, tag="ones2")
            sqc = constp.tile([P, NJOBS, MT], mybir.dt.float32, tag="sqc")

            nc.sync.dma_start(sq2[:], sq2_d[:])
            nc.sync.dma_start(ones2[:], ones_d[:])
            nc.sync.dma_start(sqc[:], sqc_d[:])
            # Operand loads split per (job, k) and issued job-major so the
            # first job's matmuls can start after ~2.5MB instead of 12.5MB.
            for j in range(NJOBS):
                for k in range(KT):
                    nc.sync.dma_start(lhs[:, k, j], lhs_d[k, :, j])
                    nc.sync.dma_start(rhs[:, k, j], rhs_d[k, :, j])

            for j in range(NJOBS):
                for m in range(MT):
                    ps = psump.tile([P, NSUP], mybir.dt.float32, tag="ps")
                    # k outer / bank inner: 4 consecutive matmuls share one
                    # stationary operand and pipeline after LDW dedup.
                    for k in range(KT):
                        for b in range(NB):
                            nc.tensor.matmul(
                                ps[:, b * NSUB:(b + 1) * NSUB],
                                lhs[:, k, j, m * P:(m + 1) * P],
                                rhs[:, k, j, b * NSUB:(b + 1) * NSUB],
                                start=(k == 0),
                                stop=False,
                            )
                    for b in range(NB):
                        nc.tensor.matmul(
                            ps[:, b * NSUB:(b + 1) * NSUB],
                            ones2[:],
                            sq2[:, j, b * NSUB:(b + 1) * NSUB],
                            start=False,
                            stop=True,
                        )
                    st = stagep.tile([P, NSUP], mybir.dt.float32, tag="st")
                    ob = stagep.tile([P, NSUP], mybir.dt.bfloat16, tag="ob")
                    nc.vector.tensor_scalar(
                        st[:], ps[:], sqc[:, j, m:m + 1], 0.0,
                        mybir.AluOpType.add, mybir.AluOpType.max,
                    )
                    nc.scalar.activation(
                        ob[:], st[:], mybir.ActivationFunctionType.Sqrt,
                    )
                    nc.sync.dma_start(out_d[j, m * P:(m + 1) * P, :], ob[:])

    nc.compile()
    _dedup_ldweights(nc)
    return nc


def _prep_inputs(mapping):
    """Host-side shard/layout: per-core concatenated job operands."""
    bf16 = ml_dtypes.bfloat16

    tbf = np.ascontiguousarray(mapping.T).astype(bf16)          # [D, N]
    tf32 = tbf.astype(np.float32)
    tneg = (tf32 * -2.0).astype(bf16)                           # exact -2x
    sq = np.sum(tf32 * tf32, axis=0, dtype=np.float32)          # [N]
    sq_hi = sq.astype(bf16)
    sq_lo = (sq - sq_hi.astype(np.float32)).astype(bf16)
    ones2 = np.ones((2, P), dtype=bf16)

    tbf_k = tbf.reshape(KT, P, N)
    tneg_k = tneg.reshape(KT, P, N)

    in_maps = []
    for c in range(NCORES):
        jobs = _jobs_for_core(c)
        rhs = np.empty((KT, P, NJOBS, NSUP), dtype=bf16)
        lhs = np.empty((KT, P, NJOBS, SW), dtype=bf16)
        sq2 = np.empty((2, NJOBS, NSUP), dtype=bf16)
        sqc = np.empty((P, NJOBS, MT), dtype=np.float32)
        for j, (s, b) in enumerate(jobs):
            rhs[:, :, j, :] = tbf_k[:, :, b * NSUP:(b + 1) * NSUP]
            lhs[:, :, j, :] = tneg_k[:, :, s * SW:(s + 1) * SW]
            sq2[0, j] = sq_hi[b * NSUP:(b + 1) * NSUP]
            sq2[1, j] = sq_lo[b * NSUP:(b + 1) * NSUP]
            sqc[:, j, :] = sq[s * SW:(s + 1) * SW].reshape(MT, P).T
        in_maps.append({
            "rhs": rhs, "lhs": lhs, "sq2": sq2, "ones2": ones2, "sqc": sqc,
        })
    return in_maps


def _assemble(results):
    """Scatter per-core job blocks and mirror the lower triangle."""
    out = np.empty((N, N), dtype=np.float32)
    for c in range(NCORES):
        blocks = results[c]["out"]                              # [NJOBS, SW, NSUP] bf16
        for j, (s, b) in enumerate(_jobs_for_core(c)):
            out[s * SW:(s + 1) * SW, b * NSUP:(b + 1) * NSUP] = \
                blocks[j].astype(np.float32)
    # rows of stripe s below the 2048-aligned cover come from the transpose
    for s in range(NSTRIPES):
        c0 = (s // 4) * NSUP
        if c0:
            out[s * SW:(s + 1) * SW, :c0] = out[:c0, s * SW:(s + 1) * SW].T
    return out


def kernel(mapping: np.ndarray) -> np.ndarray:
    from concourse.bass_utils import run_bass_kernel_spmd

    global _compiled
    mapping = np.asarray(mapping, dtype=np.float32)
    assert mapping.shape == (N, D)
    if _compiled is None:
        _compiled = _build()
    in_maps = _prep_inputs(mapping)
    res = run_bass_kernel_spmd(_compiled, in_maps, list(range(NCORES)))
    return _assemble(res.results)
